# revision 15
# baseline (speedup 1.0000x reference)
"""Trainium2 Bass kernel for nn_CLA_EP_40613210751435 (gnn_message_passing).

Sharding: data-parallel over batch B=8 across 8 NeuronCores (core c <- batch
element c); ~90M params replicated per core in bf16. No collectives.

Per-core layout: feature-major ([D, rows]); edge tensors [768 (6x128 ptiles),
4096 slots], slot = i*64 + j. All linears run as lhsT=W-tile matmuls
accumulating K in PSUM. LN stats over the feature (=partition) dim use
ones[128,128] matmuls whose M=128 output replicates the column stats across
all partitions for free. Masks come from one DMA-broadcast [128,4096] tile.
i/j broadcasts and j-reductions are step-0 / grouped free-dim APs on DVE.

Host side: LN gains folded into adjacent weights; attention q-scale folded
into Wq; v-bias applied at the o_T eviction (softmax rows sum to 1);
symmetrization 0.5 folded into fc_out l2; the tiny timestep/class embedding
is computed in numpy. These folds were validated to ~9e-7 rel-l2 against an
f64 port of the reference.

Workaround: this walrus build accepts at most ONE semaphore wait per
instruction, while Tile attaches several. split_multiwait() moves extra
waits onto same-engine NoOps inserted right before the owning instruction
(engines execute their stream in order, so the wait conjunction holds).
"""
import contextlib
import os
import sys
import numpy as np
import ml_dtypes

for _p in ("/opt/trn_rl_repo", "/root/.axon_site/_ro/trn_rl_repo"):
    if os.path.isdir(_p) and _p not in sys.path:
        sys.path.append(_p)

import concourse.bass as bass
import concourse.mybir as mybir
import concourse.tile as tile
from concourse.bass_utils import run_bass_kernel_spmd
from concourse.masks import make_identity

F32 = mybir.dt.float32
BF16 = mybir.dt.bfloat16
AF = mybir.ActivationFunctionType
ALU = mybir.AluOpType
AX = mybir.AxisListType
BF = ml_dtypes.bfloat16

D = 768
NH = 12
DH = 64
N_TF = 12
N_GNN = 6
CAT = 128
FF = 1024
B, N = 8, 64
NN = N * N
KP = D // 128          # 6 feature ptiles
FP = FF // 128         # 8
NCH = 8                # edge chunks
CW = NN // NCH         # 512
IG = N // NCH          # 8 i-groups per chunk
EPS = 1e-5
P = 128

DEBUG_TAPS = bool(int(os.environ.get("BASS_KERNEL_DEBUG", "0")))
STAGES = os.environ.get("BASS_KERNEL_STAGES", "full")  # emb|tf|gnn|full
NTF = int(os.environ.get("BASS_KERNEL_NTF", str(N_TF)))
TRACE = bool(int(os.environ.get("BASS_KERNEL_TRACE", "0")))
TFMODE = os.environ.get("BASS_KERNEL_TFMODE", "full")  # full|ffn|attn|ln


# ======================================================================
# Host-side preprocessing (folds validated vs f64 port of the reference)
# ======================================================================

def _np(x):
    return np.asarray(x, dtype=np.float32)


def _sincos(t, dim, max_period=10000.0):
    half = dim // 2
    freqs = np.exp(-np.log(max_period) * np.arange(half, dtype=np.float32) / half)
    args = t.astype(np.float32)[:, None] * freqs
    return np.concatenate([np.cos(args), np.sin(args)], axis=-1)


def _ln_np(x, g, b, eps=EPS):
    m = x.mean(-1, keepdims=True)
    v = ((x - m) ** 2).mean(-1, keepdims=True)
    return (x - m) / np.sqrt(v + eps) * g + b


def _silu_np(x):
    return x / (1.0 + np.exp(-x))


def _mlp_np(p, x):
    h = _silu_np(_ln_np(x @ _np(p["l1"]["w"]) + _np(p["l1"]["b"]),
                        _np(p["g"]), _np(p["be"])))
    return h @ _np(p["l2"]["w"]) + _np(p["l2"]["b"])


def preprocess(params):
    Pd = {}

    def lin(dst, p, wscale=1.0, gfold=None, bfold=None):
        w = _np(p["w"])
        b = _np(p["b"])
        if gfold is not None:
            w2 = w * gfold[:, None]
            b2 = bfold @ w + b
        else:
            w2, b2 = w, b
        Pd[dst + "_w"] = (w2 * wscale).astype(np.float32)
        Pd[dst + "_b"] = (b2 * wscale).astype(np.float32)

    def mlp(dst, p):
        lin(dst + "_l1", p["l1"])
        Pd[dst + "_g"] = _np(p["g"])
        Pd[dst + "_be"] = _np(p["be"])
        lin(dst + "_l2", p["l2"])

    mlp("surfz", params["surfz"])
    mlp("surfp", params["surfp"])
    mlp("edgep", params["edgep"])
    mlp("fc_out2", params["fc_out2"])
    mlp("fc_pool", params["fc_pool"])
    mlp("fc_out", params["fc_out"])
    Pd["fc_out_l2_w"] = Pd["fc_out_l2_w"] * 0.5
    Pd["fc_out_l2_b"] = Pd["fc_out_l2_b"] * 0.5

    sc = 1.0 / np.sqrt(DH)
    for li, p in enumerate(params["tf"]):
        g1, b1 = _np(p["g1"]), _np(p["b1"])
        g2, b2 = _np(p["g2"]), _np(p["b2"])
        lin(f"tf{li}_q", p["attn"]["q"], wscale=sc, gfold=g1, bfold=b1)
        lin(f"tf{li}_k", p["attn"]["k"], gfold=g1, bfold=b1)
        lin(f"tf{li}_v", p["attn"]["v"], gfold=g1, bfold=b1)
        lin(f"tf{li}_o", p["attn"]["o"])
        lin(f"tf{li}_f1", p["f1"], gfold=g2, bfold=b2)
        lin(f"tf{li}_f2", p["f2"])

    for li, p in enumerate(params["gnn"]):
        blk = p["block"]
        Pd[f"gnn{li}_eps1p"] = np.float32(1.0 + float(_np(blk["eps"])))
        Pd[f"gnn{li}_gn1_g"] = _np(blk["gn1"])
        Pd[f"gnn{li}_gn1_b"] = _np(blk["bn1"])
        lin(f"gnn{li}_gine1", blk["gine1"])
        lin(f"gnn{li}_gine2", blk["gine2"])
        lin(f"gnn{li}_q", blk["attn"]["q"], wscale=sc)
        lin(f"gnn{li}_k", blk["attn"]["k"])
        lin(f"gnn{li}_v", blk["attn"]["v"])
        lin(f"gnn{li}_o", blk["attn"]["o"])
        gn2, bn2 = _np(blk["gn2"]), _np(blk["bn2"])
        lin(f"gnn{li}_f1", blk["f1"], gfold=gn2, bfold=bn2)
        lin(f"gnn{li}_f2", blk["f2"])
        ge, bee = _np(blk["ge"]), _np(blk["bee"])
        lin(f"gnn{li}_e1", blk["e1"], gfold=ge, bfold=bee)
        lin(f"gnn{li}_e2", blk["e2"])
        lin(f"gnn{li}_linf", p["lin_f"])
        lin(f"gnn{li}_line", p["lin_e"])

    gtf, btf = _np(params["tf_norm"]["g"]), _np(params["tf_norm"]["b"])
    W1p = Pd["fc_pool_l1_w"]
    Pd["fc_pool_G_w"] = (W1p * gtf[:, None]).astype(np.float32)
    Pd["fc_pool_G_ch"] = (0.5 * (btf @ W1p)).astype(np.float32)
    Pd["fc_out2_l1_w_dez"] = Pd["fc_out2_l1_w"][:D].copy()
    Pd["fc_out2_l1_w_eh"] = Pd["fc_out2_l1_w"][D:2 * D].copy()
    Pd["fc_out2_l1_w_ls"] = Pd["fc_out2_l1_w"][2 * D:].copy()
    Pd["class_embed"] = _np(params["class_embed"])
    return Pd


def host_t_vec(Pd, params, timesteps, class_label):
    emb = _sincos(np.asarray(timesteps), D)
    t = _mlp_np(params["time"], emb)
    return (t + Pd["class_embed"][np.asarray(class_label)]).astype(np.float32)


class Smalls:
    """All per-feature vectors packed as f32 columns of one [128, C] array."""

    def __init__(self):
        self.cols = []
        self.index = {}

    def add(self, name, vec):
        vec = np.asarray(vec, np.float32).reshape(-1)
        ncol = max(1, (len(vec) + P - 1) // P)
        pad = np.zeros(ncol * P, np.float32)
        pad[:len(vec)] = vec
        self.index[name] = len(self.cols)
        for c in range(ncol):
            self.cols.append(pad[c * P:(c + 1) * P])

    def array(self):
        return np.ascontiguousarray(np.stack(self.cols, axis=1))


# ======================================================================
# Device program
# ======================================================================

class Prog:
    def __init__(self, smalls_index, smalls_ncols, bias_iszero, eps1p_host,
                 wshapes):
        self.nc = bass.Bass()
        self.dram = {}
        self.w_needed = []
        self.smalls_index = smalls_index
        self.smalls_ncols = smalls_ncols
        self.bias_iszero = bias_iszero
        self.eps1p_host = eps1p_host
        self.wshapes = wshapes
        self.taps = {}

    def wparam(self, name):
        return self.param(name, self.wshapes[name], BF16)

    def param(self, name, shape, dtype):
        if name not in self.dram:
            self.dram[name] = self.nc.declare_dram_parameter(
                name, list(shape), dtype, isOutput=False)
            self.w_needed.append((name, tuple(shape), dtype))
        return self.dram[name]

    def build(self):
        nc = self.nc
        self.out = nc.declare_dram_parameter("out", [6, NN], F32, isOutput=True)
        self.dez_dram = nc.dram_tensor("dez_scratch", [KP, P, NN], BF16)
        self.eh_dram = nc.dram_tensor("eh_scratch", [KP, P, NN], BF16)
        self.ep_dram = nc.dram_tensor("ep_scratch", [KP, P, NN], BF16)
        self.hs_dram = nc.dram_tensor("hs_scratch", [KP, P, NN], BF16)
        self.ep3_dram = nc.dram_tensor("ep3_scratch", [KP, P, NN], BF16)

        with tile.TileContext(nc) as tc, contextlib.ExitStack() as st:
            self.tc = tc
            self.const = st.enter_context(tc.tile_pool(name="const", bufs=1))
            self.big = st.enter_context(tc.tile_pool(name="big", bufs=7))
            self.wp = st.enter_context(tc.tile_pool(name="wp", bufs=13))
            self.wps = st.enter_context(tc.tile_pool(name="wps", bufs=8))
            self.nd = st.enter_context(tc.tile_pool(name="nd", bufs=4))
            self.wk = st.enter_context(tc.tile_pool(name="wk", bufs=2))
            self.ps = st.enter_context(tc.tile_pool(name="ps", bufs=3,
                                                    space="PSUM"))
            self.ps_st = st.enter_context(tc.tile_pool(name="ps_st", bufs=1,
                                                       space="PSUM"))
            self._consts()
            self._embeddings()
            if STAGES == "emb":
                self._tap("e_emb", self.e, NN)
                self._dummy_out()
                return self
            x = self._tf_stack()
            self.x_tf = x
            self._tap("x_tf", x, N)
            if STAGES == "tf":
                self._dummy_out()
                return self
            h = self._gnn_stack()
            self._tap("h_gnn", h, N)
            self._tap("e_fin", self.e, NN)
            self._tap("fh_fin", self.fh, N)
            if STAGES == "gnn":
                self._dummy_out()
                return self
            self._finale(h)
        return self

    def _dummy_out(self):
        nc = self.nc
        z = self.wk.tile([6, NN], F32, tag="outc", bufs=1, name="dummy_out")
        nc.vector.memset(z[:], 0.0)
        nc.sync.dma_start(out=self.out[:], in_=z[:])

    # ---------------- helpers ----------------
    def _tap(self, nm, tiles, cols):
        if not DEBUG_TAPS:
            return
        nc = self.nc
        dt = tiles[0].dtype
        tap = self.nc.declare_dram_parameter(
            "tap_" + nm, [len(tiles), P, cols], dt, isOutput=True)
        self.taps[nm] = (tap, dt)
        for kp in range(len(tiles)):
            nc.sync.dma_start(out=tap[kp], in_=tiles[kp][:, :cols])

    def scol(self, name, idx=0, rows=P):
        c = self.smalls_index[name] + idx
        return self.smalls_sb[:rows, c:c + 1]

    def wtile(self, name, kp, pool=None, bufs=None):
        nc = self.nc
        w = self.wparam(name)
        din, dout = w.shape
        pool = pool or self.wp
        if din <= P:
            t = pool.tile([din, dout], BF16, tag="w", bufs=bufs or 13,
                          name=f"w_{name}")
            nc.sync.dma_start(out=t[:], in_=w[:])
        else:
            t = pool.tile([P, dout], BF16, tag="w", bufs=bufs or 13,
                          name=f"w_{name}_{kp}")
            nc.sync.dma_start(out=t[:], in_=w[kp * P:(kp + 1) * P])
        return t

    def load_w(self, name):
        din = self.wparam(name).shape[0]
        nkp = (din + P - 1) // P
        return [self.wtile(name, kp) for kp in range(nkp)]

    def mm_feature(self, wname, x_tiles, ncols, bias=None, act=None,
                   out_dtype=BF16, out_tag="mmo", out_bufs=8, gb_pre=None,
                   out_pool=None):
        """act(W.T @ x + b): returns list of Dout/128 tiles [P, ncols]."""
        nc = self.nc
        w = self.wparam(wname)
        din, dout = w.shape
        nkp = (din + P - 1) // P
        nop = (dout + P - 1) // P
        wt = [self.wtile(wname, kp) for kp in range(nkp)]
        outs = []
        for op in range(nop):
            mw = min(P, dout - op * P)
            ps = self.ps.tile([P, ncols], F32, tag="mmps", bufs=3,
                              name="mmps")
            for kp in range(nkp):
                kw = min(P, din - kp * P)
                nc.tensor.matmul(
                    ps[:mw, :], wt[kp][:kw, op * P:op * P + mw],
                    x_tiles[kp][:kw, :ncols],
                    start=(kp == 0), stop=(kp == nkp - 1))
            o = (out_pool or self.nd).tile([P, ncols], out_dtype,
                                           tag=out_tag, bufs=out_bufs,
                                           name=out_tag)
            bias_ap = self.scol(bias, op) if bias is not None else 0.0
            if act == "relu":
                nc.scalar.activation(o[:mw, :], ps[:mw, :], AF.Relu,
                                     bias=bias_ap)
            elif act == "silu":
                nc.scalar.activation(o[:mw, :], ps[:mw, :], AF.Silu,
                                     bias=bias_ap)
            elif act == "silu_gb":
                nc.scalar.activation(o[:mw, :], ps[:mw, :], AF.Silu,
                                     bias=self.scol(gb_pre + "_be", op),
                                     scale=self.scol(gb_pre + "_g", op))
            else:
                nc.scalar.activation(o[:mw, :], ps[:mw, :], AF.Identity,
                                     bias=bias_ap)
            outs.append(o)
        return outs

    def ln_stats(self, x_tiles, ncols, sq_tiles):
        """Feature-dim LN stats, partition-replicated.
        x_tiles bf16 [KP][P, ncols]; sq_tiles: scratch (overwritten).
        Returns (m bf16, rs bf16) [P, ncols]."""
        nc = self.nc
        ps_s = self.ps_st.tile([P, ncols], F32, tag="lnsum", bufs=1,
                               name="lnsum")
        ps_q = self.ps_st.tile([P, ncols], F32, tag="lnsq", bufs=1,
                               name="lnsq")
        for kp in range(KP):
            nc.scalar.activation(sq_tiles[kp][:, :ncols],
                                 x_tiles[kp][:, :ncols], AF.Square)
        for kp in range(KP):
            nc.tensor.matmul(ps_s[:], self.ones_bf[:],
                             x_tiles[kp][:, :ncols],
                             start=(kp == 0), stop=(kp == KP - 1))
        for kp in range(KP):
            nc.tensor.matmul(ps_q[:], self.ones_bf[:],
                             sq_tiles[kp][:, :ncols],
                             start=(kp == 0), stop=(kp == KP - 1))
        m_f = self.wk.tile([P, ncols], F32, tag="st_mf", bufs=1, name="m_f")
        m_bf = self.wk.tile([P, ncols], BF16, tag="st_m", bufs=2, name="m_bf")
        var = self.wk.tile([P, ncols], F32, tag="st_var", bufs=1, name="var")
        rs_f = self.wk.tile([P, ncols], F32, tag="st_rsf", bufs=1, name="rs_f")
        rs_bf = self.wk.tile([P, ncols], BF16, tag="st_rs", bufs=2,
                             name="rs_bf")
        nc.vector.tensor_scalar_mul(m_f[:], ps_s[:], 1.0 / D)
        nc.vector.tensor_copy(m_bf[:], m_f[:])
        nc.vector.tensor_scalar_mul(var[:], ps_q[:], 1.0 / D)
        nc.vector.tensor_mul(rs_f[:], m_f[:], m_f[:])       # m^2 (scratch)
        nc.vector.tensor_sub(var[:], var[:], rs_f[:])
        nc.scalar.activation(rs_f[:], var[:], AF.Sqrt,
                             bias=self.scol("eps_col"))
        nc.vector.reciprocal(rs_f[:], rs_f[:])
        nc.vector.tensor_copy(rs_bf[:], rs_f[:])
        return m_bf, rs_bf

    def ln_norm(self, x_tiles, ncols, tag, bufs=8):
        """Full LN to z (no affine): allocates z tiles, uses them as the
        square scratch first."""
        nc = self.nc
        z = [self.wk.tile([P, ncols], BF16, tag=tag, bufs=bufs, name=tag)
             for _ in range(KP)]
        m, rs = self.ln_stats(x_tiles, ncols, z)
        for kp in range(KP):
            nc.vector.tensor_sub(z[kp][:, :ncols], x_tiles[kp][:, :ncols],
                                 m[:, :ncols])
            nc.vector.tensor_mul(z[kp][:, :ncols], z[kp][:, :ncols],
                                 rs[:, :ncols])
        return z

    def node_ln(self, x_tiles, tag):
        """LN of node-level tiles (casts f32 residual to bf16 first)."""
        nc = self.nc
        if x_tiles[0].dtype != BF16:
            xb = [self.nd.tile([P, N], BF16, tag="lncast", bufs=8,
                               name="lncast") for _ in range(KP)]
            for kp in range(KP):
                nc.vector.tensor_copy(xb[kp][:], x_tiles[kp][:, :N])
            x_tiles = xb
        return self.ln_norm(x_tiles, N, tag)

    # ---------------- consts ----------------
    def _consts(self):
        nc = self.nc
        self.smalls_dram = self.param("smalls", (P, self.smalls_ncols), F32)
        self.smalls_sb = self.const.tile([P, self.smalls_ncols], F32,
                                         name="smalls_sb")
        nc.sync.dma_start(out=self.smalls_sb[:], in_=self.smalls_dram[:])
        self.tcols_dram = self.param("t_cols", (P, KP), F32)
        self.tcols = self.const.tile([P, KP], F32, name="tcols")
        nc.sync.dma_start(out=self.tcols[:], in_=self.tcols_dram[:])
        self.ones_bf = self.const.tile([P, P], BF16, name="ones_bf")
        nc.vector.memset(self.ones_bf[:], 1.0)
        self.ident_bf = self.const.tile([P, P], BF16, name="ident_bf")
        make_identity(nc, self.ident_bf)
        em_dram = self.param("em_bf16", (NN,), BF16)
        self.em = self.big.tile([P, NN], BF16, tag="big", bufs=7, name="em")
        nc.sync.dma_start(out=self.em[:],
                          in_=em_dram[:].unsqueeze(0).to_broadcast((P, NN)))

    # ---------------- embeddings ----------------
    def _node_mlp(self, pre, in_name, din):
        nc = self.nc
        xin = self.param(in_name, (din, N), BF16)
        xt = self.nd.tile([din, N], BF16, tag="nmlp_in", bufs=2,
                          name="nmlp_in")
        nc.sync.dma_start(out=xt[:], in_=xin[:])
        h = self.mm_feature(pre + "_l1_w", [xt], N, bias=pre + "_l1_b",
                            out_tag="nm1", out_bufs=7)
        z = self.ln_norm(h, N, tag="nmz", bufs=7)
        hs = [self.nd.tile([P, N], BF16, tag="nmhs", bufs=7, name="nmhs")
              for _ in range(KP)]
        for kp in range(KP):
            nc.scalar.activation(hs[kp][:], z[kp][:, :N], AF.Silu,
                                 bias=self.scol(pre + "_be", kp),
                                 scale=self.scol(pre + "_g", kp))
        return self.mm_feature(pre + "_l2_w", hs, N, bias=pre + "_l2_b",
                               out_dtype=F32, out_tag="nm2", out_bufs=7)

    def _embeddings(self):
        nc = self.nc
        sp = self._node_mlp("surfp", "surfPos_T", 6)
        sz = self._node_mlp("surfz", "surfZ_T", 48)
        self.h0 = [self.nd.tile([P, N], F32, tag="h0", bufs=7, name="h0")
                   for _ in range(KP)]
        self.se_half = [self.nd.tile([P, N], BF16, tag="seh", bufs=7,
                                     name="seh") for _ in range(KP)]
        for kp in range(KP):
            nc.vector.tensor_add(self.h0[kp][:], sp[kp][:, :N], sz[kp][:, :N])
            nc.vector.tensor_scalar_mul(self.se_half[kp][:], self.h0[kp][:],
                                        0.5)
            nc.vector.tensor_scalar_add(self.h0[kp][:], self.h0[kp][:],
                                        self.tcols[:, kp:kp + 1])
        # edgep MLP + e(=dez) init + pooled, chunked over the 4096 slots
        epos_dram = self.param("edgePos_T", (6, NN), BF16)
        self.e = [self.big.tile([P, NN], BF16, tag="big", bufs=7,
                                name=f"e{kp}") for kp in range(KP)]
        pooled = self.nd.tile([P, N * KP], F32, tag="pooled", bufs=1,
                              name="pooled")
        l2w = self.load_w("edgep_l2_w")
        for c in range(NCH):
            cs = slice(c * CW, (c + 1) * CW)
            i0 = c * IG
            ein = self.nd.tile([6, CW], BF16, tag="eposin", bufs=2,
                               name="eposin")
            nc.sync.dma_start(out=ein[:], in_=epos_dram[:, cs])
            h = self.mm_feature("edgep_l1_w", [ein], CW, bias="edgep_l1_b",
                                out_tag="finh", out_bufs=7, out_pool=self.wk)
            z = self.ln_norm(h, CW, tag="ez", bufs=7)
            hs = [self.wk.tile([P, CW], BF16, tag="h1", bufs=7, name="ephs")
                  for _ in range(KP)]
            for kp in range(KP):
                nc.scalar.activation(hs[kp][:], z[kp][:], AF.Silu,
                                     bias=self.scol("edgep_be", kp),
                                     scale=self.scol("edgep_g", kp))
            for op in range(KP):
                ps = self.ps.tile([P, CW], F32, tag="mmps", bufs=3,
                                  name="mmps")
                for kp in range(KP):
                    nc.tensor.matmul(ps[:], l2w[kp][:, op * P:(op + 1) * P],
                                     hs[kp][:], start=(kp == 0),
                                     stop=(kp == KP - 1))
                ee = self.wk.tile([P, CW], BF16, tag="eec", bufs=2, name="eec")
                nc.scalar.activation(ee[:], ps[:], AF.Identity,
                                     bias=self.scol("edgep_l2_b", op))
                eem = self.wk.tile([P, CW], BF16, tag="eemc", bufs=2,
                                   name="eem")
                nc.vector.tensor_mul(eem[:], ee[:], self.em[:, cs])
                nc.vector.reduce_sum(
                    pooled[:, op * N + i0:op * N + i0 + IG],
                    eem[:].rearrange("p (i j) -> p i j", j=N), axis=AX.X)
                seh = self.se_half[op]
                tmp = self.wk.tile([P, CW], BF16, tag="dezc", bufs=2,
                                   name="dezc")
                nc.vector.tensor_add(
                    tmp[:].rearrange("p (i j) -> p i j", j=N),
                    ee[:].rearrange("p (i j) -> p i j", j=N),
                    seh[:, i0:i0 + IG].unsqueeze(2).to_broadcast((P, IG, N)))
                nc.vector.tensor_add(
                    tmp[:].rearrange("p (i j) -> p i j", j=N),
                    tmp[:].rearrange("p (i j) -> p i j", j=N),
                    seh[:, :N].unsqueeze(1).to_broadcast((P, IG, N)))
                nc.vector.tensor_scalar_add(tmp[:], tmp[:],
                                            self.tcols[:, op:op + 1])
                nc.vector.tensor_mul(self.e[op][:, cs], tmp[:],
                                     self.em[:, cs])
        for kp in range(KP):
            nc.sync.dma_start(out=self.dez_dram[kp], in_=self.e[kp][:])
        self.x0 = [self.nd.tile([P, N], F32, tag="xres", bufs=7, name="x0")
                   for _ in range(KP)]
        for kp in range(KP):
            nc.vector.tensor_scalar_mul(pooled[:, kp * N:(kp + 1) * N],
                                        pooled[:, kp * N:(kp + 1) * N],
                                        1.0 / N)
            nc.vector.tensor_add(self.x0[kp][:], self.h0[kp][:],
                                 pooled[:, kp * N:(kp + 1) * N])
        self._tap("x0_dbg", self.x0, N)

    # ---------------- attention ----------------
    def _attention(self, pre, z_tiles):
        nc = self.nc
        q = self.mm_feature(pre + "_q_w", z_tiles, N, bias=pre + "_q_b",
                            out_tag="att_q", out_bufs=7)
        k = self.mm_feature(pre + "_k_w", z_tiles, N, bias=pre + "_k_b",
                            out_tag="att_k", out_bufs=7)
        vw = self.load_w(pre + "_v_w")
        v_row = self.nd.tile([N, D], BF16, tag="vrow", bufs=1, name="vrow")
        for (n0, nw) in ((0, 512), (512, 256)):
            ps = self.ps.tile([N, nw], F32, tag="mmps", bufs=3, name="vps")
            for kp in range(KP):
                nc.tensor.matmul(ps[:], z_tiles[kp][:, :N],
                                 vw[kp][:, n0:n0 + nw],
                                 start=(kp == 0), stop=(kp == KP - 1))
            nc.scalar.activation(v_row[:, n0:n0 + nw], ps[:], AF.Copy)
        # per-head psum tiles: PSUM matmul writes must be tile-offset-0 /
        # bank-aligned (column-sliced writes into one bank crash the device)
        s_sb = self.nd.tile([N, D], F32, tag="ssb", bufs=1, name="s_sb")
        for h in range(NH):
            kp, off = h // 2, (h % 2) * DH
            psh = self.ps.tile([N, DH], F32, tag="hps", bufs=2, name="psh")
            nc.tensor.matmul(psh[:], q[kp][off:off + DH, :N],
                             k[kp][off:off + DH, :N], start=True, stop=True)
            nc.scalar.activation(s_sb[:, h * DH:(h + 1) * DH], psh[:],
                                 AF.Copy)
        negmax = self.nd.tile([N, NH], F32, tag="negmax", bufs=2,
                              name="negmax")
        nc.vector.reduce_max(negmax[:],
                             s_sb[:].rearrange("p (h j) -> p h j", j=N),
                             axis=AX.X)
        nc.vector.tensor_scalar_mul(negmax[:], negmax[:], -1.0)
        p_sb = self.nd.tile([N, D], BF16, tag="psoft", bufs=1, name="p_sb")
        den = self.nd.tile([N, NH], F32, tag="den", bufs=2, name="den")
        for h in range(NH):
            nc.scalar.activation(p_sb[:, h * DH:(h + 1) * DH],
                                 s_sb[:, h * DH:(h + 1) * DH], AF.Exp,
                                 bias=negmax[:, h:h + 1],
                                 accum_out=den[:, h:h + 1])
        rden = self.nd.tile([N, NH], F32, tag="rden", bufs=2, name="rden")
        nc.vector.reciprocal(rden[:], den[:])
        for h in range(NH):
            nc.vector.tensor_scalar_mul(p_sb[:, h * DH:(h + 1) * DH],
                                        p_sb[:, h * DH:(h + 1) * DH],
                                        rden[:, h:h + 1])
        pT = self.nd.tile([N, D], BF16, tag="pT", bufs=1, name="pT")
        for h in range(NH):
            psh = self.ps.tile([N, DH], BF16, tag="hps", bufs=2, name="psh")
            nc.tensor.transpose(psh[:], p_sb[:, h * DH:(h + 1) * DH],
                                self.ident_bf[:N, :N])
            nc.scalar.activation(pT[:, h * DH:(h + 1) * DH], psh[:], AF.Copy)
        o_T = []
        for op in range(KP):
            ps = self.ps.tile([P, N], F32, tag="mmps", bufs=3, name="ops")
            for sub in range(2):
                h = op * 2 + sub
                nc.tensor.matmul(ps[sub * DH:(sub + 1) * DH, :],
                                 v_row[:, h * DH:(h + 1) * DH],
                                 pT[:, h * DH:(h + 1) * DH],
                                 start=True, stop=True)
            o = self.nd.tile([P, N], BF16, tag="oT", bufs=7, name="oT")
            # v-bias folded here: softmax rows sum to 1
            nc.scalar.activation(o[:], ps[:], AF.Identity,
                                 bias=self.scol(pre + "_v_b", op))
            o_T.append(o)
        return self.mm_feature(pre + "_o_w", o_T, N, bias=pre + "_o_b",
                               out_tag="attno", out_bufs=7)

    # ---------------- TF stack ----------------
    def _tf_stack(self):
        nc = self.nc
        x = self.x0
        for li in range(NTF):
            pre = f"tf{li}"
            z = self.node_ln(x, tag="tfz")
            if TFMODE == "ln":
                continue
            if TFMODE in ("full", "attn"):
                att = self._attention(pre, z)
                for kp in range(KP):
                    nc.vector.tensor_add(x[kp][:], x[kp][:], att[kp][:, :N])
            if TFMODE in ("full", "ffn"):
                z2 = self.node_ln(x, tag="tfz2")
                hh = self.mm_feature(pre + "_f1_w", z2, N, bias=pre + "_f1_b",
                                     act="relu", out_tag="ff1",
                                     out_bufs=FP + 1)
                f2 = self.mm_feature(pre + "_f2_w", hh, N,
                                     bias=pre + "_f2_b", out_tag="ff2",
                                     out_bufs=7)
                for kp in range(KP):
                    nc.vector.tensor_add(x[kp][:], x[kp][:], f2[kp][:, :N])
        return x

    # ---------------- GNN stack ----------------
    def _gnn_stack(self):
        nc = self.nc
        h = self.h0
        self.fh = [self.nd.tile([P, N], BF16, tag="fh", bufs=7,
                                name=f"fh{kp}") for kp in range(KP)]
        for li in range(N_GNN):
            pre = f"gnn{li}"
            zc = self.node_ln(h, tag="gz")
            xn = [self.nd.tile([P, N], BF16, tag="xn", bufs=7, name="xn")
                  for _ in range(KP)]
            for kp in range(KP):
                nc.scalar.activation(xn[kp][:], zc[kp][:, :N], AF.Identity,
                                     bias=self.scol(pre + "_gn1_b", kp),
                                     scale=self.scol(pre + "_gn1_g", kp))
            # agg[d, i] = sum_j relu(xn[d,j] + e[d,(i,j)]) * em  (chunked)
            agg = [self.nd.tile([P, N], BF16, tag="agg", bufs=7, name="agg")
                   for _ in range(KP)]
            for c in range(NCH):
                cs = slice(c * CW, (c + 1) * CW)
                i0 = c * IG
                for kp in range(KP):
                    t1 = self.wk.tile([P, CW], BF16, tag="msg", bufs=3,
                                      name="msg")
                    nc.vector.tensor_add(
                        t1[:].rearrange("p (i j) -> p i j", j=N),
                        self.e[kp][:, cs].rearrange("p (i j) -> p i j", j=N),
                        xn[kp][:, :N].unsqueeze(1).to_broadcast((P, IG, N)))
                    nc.scalar.activation(t1[:], t1[:], AF.Relu)
                    nc.vector.tensor_mul(t1[:], t1[:], self.em[:, cs])
                    with nc.allow_low_precision(
                            reason="64-term bf16 agg sum; values O(10)"):
                        nc.vector.reduce_sum(
                            agg[kp][:, i0:i0 + IG],
                            t1[:].rearrange("p (i j) -> p i j", j=N),
                            axis=AX.X)
            u = [self.nd.tile([P, N], BF16, tag="ug", bufs=7, name="ug")
                 for _ in range(KP)]
            for kp in range(KP):
                nc.vector.tensor_scalar_mul(u[kp][:], xn[kp][:],
                                            self.eps1p_host[li])
                nc.vector.tensor_add(u[kp][:], u[kp][:], agg[kp][:])
            g1 = self.mm_feature(pre + "_gine1_w", u, N,
                                 bias=pre + "_gine1_b", act="silu",
                                 out_tag="gg1", out_bufs=7)
            local = self.mm_feature(pre + "_gine2_w", g1, N,
                                    bias=pre + "_gine2_b", out_tag="gg2",
                                    out_bufs=7)
            glob = self._attention(pre, xn)
            for kp in range(KP):
                nc.vector.tensor_add(h[kp][:], h[kp][:], local[kp][:, :N])
                nc.vector.tensor_add(h[kp][:], h[kp][:], glob[kp][:, :N])
            z2 = self.node_ln(h, tag="gz2")
            f1 = self.mm_feature(pre + "_f1_w", z2, N, bias=pre + "_f1_b",
                                 act="silu", out_tag="gf1", out_bufs=FP + 1)
            f2 = self.mm_feature(pre + "_f2_w", f1, N, bias=pre + "_f2_b",
                                 out_tag="gf2", out_bufs=7)
            for kp in range(KP):
                nc.vector.tensor_add(h[kp][:], h[kp][:], f2[kp][:, :N])
            # ---- edge update (chunked) ----
            hb = [self.nd.tile([P, N], BF16, tag="hb", bufs=7, name="hb")
                  for _ in range(KP)]
            for kp in range(KP):
                nc.vector.tensor_copy(hb[kp][:], h[kp][:])
            e1w = self.load_w(pre + "_e1_w")
            e2w = self.load_w(pre + "_e2_w")
            lew = [self.wtile(pre + "_line_w", kp, pool=self.wps, bufs=8)
                   for kp in range(KP)]
            b2_zero = self.bias_iszero.get(pre + "_e2_b", False)
            ble_zero = self.bias_iszero.get(pre + "_line_b", False)
            for c in range(NCH):
                cs = slice(c * CW, (c + 1) * CW)
                i0 = c * IG
                pair = [self.wk.tile([P, CW], BF16, tag="pair", bufs=7,
                                     name="pair") for _ in range(KP)]
                for kp in range(KP):
                    nc.vector.tensor_add(
                        pair[kp][:].rearrange("p (i j) -> p i j", j=N),
                        self.e[kp][:, cs].rearrange("p (i j) -> p i j", j=N),
                        hb[kp][:, i0:i0 + IG].unsqueeze(2)
                        .to_broadcast((P, IG, N)))
                    nc.vector.tensor_add(
                        pair[kp][:].rearrange("p (i j) -> p i j", j=N),
                        pair[kp][:].rearrange("p (i j) -> p i j", j=N),
                        hb[kp][:, :N].unsqueeze(1).to_broadcast((P, IG, N)))
                z = self.ln_norm(pair, CW, tag="ez", bufs=7)
                h1 = []
                for op in range(KP):
                    ps = self.ps.tile([P, CW], F32, tag="mmps", bufs=3,
                                      name="mmps")
                    for kp in range(KP):
                        nc.tensor.matmul(ps[:],
                                         e1w[kp][:, op * P:(op + 1) * P],
                                         z[kp][:], start=(kp == 0),
                                         stop=(kp == KP - 1))
                    t = self.wk.tile([P, CW], BF16, tag="h1", bufs=7,
                                     name="h1")
                    nc.scalar.activation(t[:], ps[:], AF.Silu,
                                         bias=self.scol(pre + "_e1_b", op))
                    h1.append(t)
                for op in range(KP):
                    ps = self.ps.tile([P, CW], F32, tag="mmps", bufs=3,
                                      name="mmps")
                    for kp in range(KP):
                        nc.tensor.matmul(ps[:],
                                         e2w[kp][:, op * P:(op + 1) * P],
                                         h1[kp][:], start=(kp == 0),
                                         stop=(kp == KP - 1))
                    t2 = self.wk.tile([P, CW], BF16, tag="e2t", bufs=3,
                                      name="e2t")
                    if b2_zero:
                        nc.vector.tensor_mul(t2[:], ps[:], self.em[:, cs])
                    else:
                        nc.scalar.activation(t2[:], ps[:], AF.Identity,
                                             bias=self.scol(pre + "_e2_b",
                                                            op))
                        nc.vector.tensor_mul(t2[:], t2[:], self.em[:, cs])
                    nc.vector.tensor_add(self.e[op][:, cs],
                                         self.e[op][:, cs], t2[:])
                # lin_e on the updated e chunk -> eh (masked) -> DRAM
                ps = self.ps.tile([P, CW], F32, tag="mmps", bufs=3,
                                  name="mmps")
                for kp in range(KP):
                    nc.tensor.matmul(ps[:], lew[kp][:],
                                     self.e[kp][:, cs], start=(kp == 0),
                                     stop=(kp == KP - 1))
                el = self.wk.tile([P, CW], BF16, tag="ehc", bufs=3,
                                  name="ehc")
                if ble_zero:
                    nc.vector.tensor_mul(el[:], ps[:], self.em[:, cs])
                else:
                    nc.scalar.activation(el[:], ps[:], AF.Identity,
                                         bias=self.scol(pre + "_line_b", 0))
                    nc.vector.tensor_mul(el[:], el[:], self.em[:, cs])
                nc.sync.dma_start(out=self.eh_dram[li][:, cs], in_=el[:])
            # lin_f -> fh[li]
            lfw = [self.wtile(pre + "_linf_w", kp, pool=self.wps, bufs=8)
                   for kp in range(KP)]
            ps = self.ps.tile([P, N], F32, tag="mmps", bufs=3, name="mmps")
            for kp in range(KP):
                nc.tensor.matmul(ps[:], lfw[kp][:], hb[kp][:],
                                 start=(kp == 0), stop=(kp == KP - 1))
            nc.scalar.activation(self.fh[li][:], ps[:], AF.Identity,
                                 bias=self.scol(pre + "_linf_b", 0))
        return h

    # ---------------- finale ----------------
    def _stream_in(self, dram, c, tag="sin", bufs=13):
        cs = slice(c * CW, (c + 1) * CW)
        tiles = []
        for kp in range(KP):
            t = self.wk.tile([P, CW], BF16, tag=tag, bufs=bufs,
                             name=f"{tag}{kp}")
            self.nc.sync.dma_start(out=t[:], in_=dram[kp][:, cs])
            tiles.append(t)
        return tiles

    def _edge_mlp_chunk(self, pre, hch, l2w, out_dram, c, corr_src=None):
        """(optional +corr) -> +b1 -> LN -> silu(g,be) -> l2 -> out_dram."""
        nc = self.nc
        cs = slice(c * CW, (c + 1) * CW)
        z = self.ln_norm(hch, CW, tag="ez", bufs=7)
        hs = [self.wk.tile([P, CW], BF16, tag="h1", bufs=7, name="fhs")
              for _ in range(KP)]
        for kp in range(KP):
            nc.scalar.activation(hs[kp][:], z[kp][:], AF.Silu,
                                 bias=self.scol(pre + "_be", kp),
                                 scale=self.scol(pre + "_g", kp))
        for op in range(KP):
            ps = self.ps.tile([P, CW], F32, tag="mmps", bufs=3, name="mmps")
            for kp in range(KP):
                nc.tensor.matmul(ps[:], l2w[kp][:, op * P:(op + 1) * P],
                                 hs[kp][:], start=(kp == 0),
                                 stop=(kp == KP - 1))
            o = self.wk.tile([P, CW], BF16, tag="e2t", bufs=3, name="fo")
            nc.scalar.activation(o[:], ps[:], AF.Identity,
                                 bias=self.scol(pre + "_l2_b", op))
            nc.sync.dma_start(out=out_dram[op][:, cs], in_=o[:])

    def _corr_chunk(self, Gt, op, c):
        """em * (G_i + G_j) for chunk c, from node tile Gt[op] [P, N]."""
        nc = self.nc
        cs = slice(c * CW, (c + 1) * CW)
        i0 = c * IG
        corr = self.wk.tile([P, CW], BF16, tag="corr", bufs=2, name="corr")
        nc.vector.tensor_add(
            corr[:].rearrange("p (i j) -> p i j", j=N),
            Gt[op][:, i0:i0 + IG].unsqueeze(2).to_broadcast((P, IG, N)),
            Gt[op][:, :N].unsqueeze(1).to_broadcast((P, IG, N)))
        nc.vector.tensor_mul(corr[:], corr[:], self.em[:, cs])
        return corr

    def _finale(self, h_gnn):
        nc = self.nc
        z_gs = self.node_ln(self.x_tf, tag="zgs")
        Gb = self.mm_feature("fc_pool_G_w", z_gs, N, bias="fc_pool_G_ch",
                             out_tag="Gb", out_bufs=7)
        G2b = self.mm_feature("fc_out2_l1_w_ls", self.fh, N, out_tag="G2b",
                              out_bufs=7)
        # fc_out2 pass A: l1 + corr -> LN -> silu -> hs_dram
        # (two passes keep concurrently-live weight tiles within the pool:
        #  dez 6 + eh 6 in pass A, l2's 6 in pass B)
        w_dez = self.load_w("fc_out2_l1_w_dez")
        w_eh = self.load_w("fc_out2_l1_w_eh")
        for c in range(NCH):
            cs = slice(c * CW, (c + 1) * CW)
            dz = self._stream_in(self.dez_dram, c, tag="sin")
            ehh = self._stream_in(self.eh_dram, c, tag="sin")
            hch = []
            for op in range(KP):
                ps = self.ps.tile([P, CW], F32, tag="mmps", bufs=3,
                                  name="mmps")
                for kp in range(KP):
                    nc.tensor.matmul(ps[:], w_dez[kp][:, op * P:(op + 1) * P],
                                     dz[kp][:], start=(kp == 0), stop=False)
                for kp in range(KP):
                    nc.tensor.matmul(ps[:], w_eh[kp][:, op * P:(op + 1) * P],
                                     ehh[kp][:], start=False,
                                     stop=(kp == KP - 1))
                corr = self._corr_chunk(G2b, op, c)
                th = self.wk.tile([P, CW], BF16, tag="finh", bufs=7,
                                  name="finh")
                nc.scalar.activation(th[:], ps[:], AF.Identity,
                                     bias=self.scol("fc_out2_l1_b", op))
                nc.vector.tensor_add(th[:], th[:], corr[:])
                hch.append(th)
            z = self.ln_norm(hch, CW, tag="ez", bufs=7)
            for kp in range(KP):
                hsv = self.wk.tile([P, CW], BF16, tag="h1", bufs=7,
                                   name="hsv")
                nc.scalar.activation(hsv[:], z[kp][:], AF.Silu,
                                     bias=self.scol("fc_out2_be", kp),
                                     scale=self.scol("fc_out2_g", kp))
                nc.sync.dma_start(out=self.hs_dram[kp][:, cs], in_=hsv[:])
        # fc_out2 pass B: l2
        w_o2l2 = self.load_w("fc_out2_l2_w")
        for c in range(NCH):
            cs = slice(c * CW, (c + 1) * CW)
            hss = self._stream_in(self.hs_dram, c, tag="sin")
            for op in range(KP):
                ps = self.ps.tile([P, CW], F32, tag="mmps", bufs=3,
                                  name="mmps")
                for kp in range(KP):
                    nc.tensor.matmul(ps[:], w_o2l2[kp][:, op * P:(op + 1) * P],
                                     hss[kp][:], start=(kp == 0),
                                     stop=(kp == KP - 1))
                o = self.wk.tile([P, CW], BF16, tag="e2t", bufs=3, name="fo")
                nc.scalar.activation(o[:], ps[:], AF.Identity,
                                     bias=self.scol("fc_out2_l2_b", op))
                nc.sync.dma_start(out=self.ep_dram[op][:, cs], in_=o[:])
        # fc_pool
        w_p1 = self.load_w("fc_pool_l1_w")
        w_p2 = self.load_w("fc_pool_l2_w")
        for c in range(NCH):
            epp = self._stream_in(self.ep_dram, c, tag="sin")
            hch = []
            for op in range(KP):
                ps = self.ps.tile([P, CW], F32, tag="mmps", bufs=3,
                                  name="mmps")
                for kp in range(KP):
                    nc.tensor.matmul(ps[:], w_p1[kp][:, op * P:(op + 1) * P],
                                     epp[kp][:], start=(kp == 0),
                                     stop=(kp == KP - 1))
                corr = self._corr_chunk(Gb, op, c)
                th = self.wk.tile([P, CW], BF16, tag="finh", bufs=7,
                                  name="finh")
                nc.scalar.activation(th[:], ps[:], AF.Identity,
                                     bias=self.scol("fc_pool_l1_b", op))
                nc.vector.tensor_add(th[:], th[:], corr[:])
                hch.append(th)
            self._edge_mlp_chunk("fc_pool", hch, w_p2, self.ep3_dram, c)
        # fc_out
        w_o1 = self.load_w("fc_out_l1_w")
        w_o2 = self.load_w("fc_out_l2_w")   # [768, 6]
        for c in range(NCH):
            cs = slice(c * CW, (c + 1) * CW)
            e3 = self._stream_in(self.ep3_dram, c, tag="sin")
            hch = []
            for op in range(KP):
                ps = self.ps.tile([P, CW], F32, tag="mmps", bufs=3,
                                  name="mmps")
                for kp in range(KP):
                    nc.tensor.matmul(ps[:], w_o1[kp][:, op * P:(op + 1) * P],
                                     e3[kp][:], start=(kp == 0),
                                     stop=(kp == KP - 1))
                th = self.wk.tile([P, CW], BF16, tag="finh", bufs=7,
                                  name="finh")
                nc.scalar.activation(th[:], ps[:], AF.Identity,
                                     bias=self.scol("fc_out_l1_b", op))
                hch.append(th)
            z = self.ln_norm(hch, CW, tag="ez", bufs=7)
            hs = [self.wk.tile([P, CW], BF16, tag="h1", bufs=7, name="ohs")
                  for _ in range(KP)]
            for kp in range(KP):
                nc.scalar.activation(hs[kp][:], z[kp][:], AF.Silu,
                                     bias=self.scol("fc_out_be", kp),
                                     scale=self.scol("fc_out_g", kp))
            ps = self.ps.tile([6, CW], F32, tag="mmps", bufs=3, name="mmps")
            for kp in range(KP):
                nc.tensor.matmul(ps[:], w_o2[kp][:, :6], hs[kp][:],
                                 start=(kp == 0), stop=(kp == KP - 1))
            th = self.wk.tile([6, CW], F32, tag="outc", bufs=3, name="outth")
            nc.scalar.activation(th[:], ps[:], AF.Identity,
                                 bias=self.scol("fc_out_l2_b", 0, rows=6))
            nc.vector.tensor_mul(th[:], th[:], self.em[:6, cs])
            nc.sync.dma_start(out=self.out[:, cs], in_=th[:])


# ======================================================================
# Multi-wait splitter (walrus 2026-05-04: one sem wait per instruction)
# ======================================================================

def split_multiwait(nc, max_waits=1):
    ctr = 0
    for f in nc.m.functions:
        for b in f.blocks:
            insts = list(b.instructions)
            out = []
            changed = False
            for inst in insts:
                si = inst.sync_info
                ow = list(si.on_wait) if (si is not None and si.on_wait) else []
                if len(ow) > max_waits:
                    keep = ow[:max_waits]
                    extra = ow[max_waits:]
                    for j in range(0, len(extra), max_waits):
                        ctr += 1
                        out.append(mybir.InstNoOp(
                            name=f"I-mws-{ctr}", engine=inst.engine,
                            sync_info=mybir.SyncInfo(
                                on_wait=extra[j:j + max_waits],
                                on_update=[])))
                    si.on_wait = keep
                    changed = True
                out.append(inst)
            if changed:
                b.instructions = out
    return ctr


# ======================================================================
# kernel() entry
# ======================================================================

_CACHE = {}


def _install_ntff_hook():
    """Provide antenv.axon_hooks (absent in this image) so bass_utils
    trace=True can reach the axon NTFF profiler."""
    import types
    if "antenv.axon_hooks" in sys.modules:
        return
    mod = types.ModuleType("antenv.axon_hooks")
    state = {"hook": None}

    def set_axon_ntff_profile_hook(h):
        state["hook"] = h

    def get_axon_ntff_profile_hook():
        if state["hook"] is None:
            try:
                if "/root/.axon_site" not in sys.path:
                    sys.path.insert(0, "/root/.axon_site")
                from trn_agent_boot.trn_boot import _ntff_profile_via_ctypes
                state["hook"] = _ntff_profile_via_ctypes(
                    "/opt/axon/libaxon_pjrt.so")
            except Exception:
                state["hook"] = None
        return state["hook"]

    mod.set_axon_ntff_profile_hook = set_axon_ntff_profile_hook
    mod.get_axon_ntff_profile_hook = get_axon_ntff_profile_hook
    sys.modules["antenv.axon_hooks"] = mod


def kernel(edgePos, surfPos, surfZ, params, timesteps, face_mask, edge_mask,
           class_label):
    """Full-input entry: shards batch over 8 cores, returns [8,64,64,6] f32.

    face_mask is all-ones per the input spec, so attention key-masking is a
    no-op and is omitted on device."""
    edgePos = np.asarray(edgePos, np.float32)
    surfPos = np.asarray(surfPos, np.float32)
    surfZ = np.asarray(surfZ, np.float32)
    edge_mask = np.asarray(edge_mask)
    Pd = preprocess(params)
    t_vec = host_t_vec(Pd, params, timesteps, class_label)

    sm = Smalls()
    bias_iszero = {}
    for k, v in sorted(Pd.items()):
        if v.ndim == 1:
            sm.add(k, v)
            bias_iszero[k] = bool(np.all(v == 0))
    sm.add("eps_col", np.full(P, EPS, np.float32))
    eps1p_host = [float(Pd[f"gnn{li}_eps1p"]) for li in range(N_GNN)]
    smalls_arr = sm.array()

    key = ("v1", smalls_arr.shape[1], tuple(sorted(bias_iszero.items())),
           tuple(eps1p_host), DEBUG_TAPS, STAGES, NTF, TFMODE)
    if key not in _CACHE:
        wshapes = {k: v.shape for k, v in Pd.items()
                   if isinstance(v, np.ndarray) and v.ndim == 2}
        prog = Prog(sm.index, smalls_arr.shape[1], bias_iszero, eps1p_host,
                    wshapes)
        prog.build()
        split_multiwait(prog.nc)
        _CACHE[key] = prog
    prog = _CACHE[key]

    base = {}
    per_core_names = {"em_bf16", "edgePos_T", "surfPos_T", "surfZ_T",
                      "t_cols"}
    for name, shape, dtype in prog.w_needed:
        if name in per_core_names:
            continue
        if name == "smalls":
            base[name] = smalls_arr
        else:
            arr = Pd[name]
            base[name] = np.ascontiguousarray(
                arr.astype(BF if dtype == BF16 else np.float32))

    in_maps = []
    for c in range(B):
        m = dict(base)
        m["em_bf16"] = edge_mask[c].reshape(NN).astype(BF)
        m["edgePos_T"] = np.ascontiguousarray(
            edgePos[c].reshape(NN, 6).T).astype(BF)
        m["surfPos_T"] = np.ascontiguousarray(surfPos[c].T).astype(BF)
        m["surfZ_T"] = np.ascontiguousarray(surfZ[c].T).astype(BF)
        m["t_cols"] = np.ascontiguousarray(
            t_vec[c].reshape(KP, P).T).astype(np.float32)
        in_maps.append(m)

    if TRACE:
        _install_ntff_hook()
    res = run_bass_kernel_spmd(prog.nc, in_maps, list(range(B)),
                               trace=TRACE)
    kernel._last_exec_ns = res.exec_time_ns
    outs = []
    for c in range(B):
        q = res.results[c]["out"].astype(np.float32)   # [6, 4096] masked
        q = q.T.reshape(N, N, 6)
        outs.append(q + q.transpose(1, 0, 2))          # symmetrize on host
    if DEBUG_TAPS:
        kernel._last_results = res
    return np.stack(outs)
